# revision 2
# baseline (speedup 1.0000x reference)
"""Trainium2 Bass kernel for nn_CentralAttention1 (sparse_attention), v2.

Self-contained: takes the FULL (unsharded) inputs as numpy arrays, shards
batch 8-ways across the NeuronCores (data parallel; each core gets
batch/8 rows of each of the 3 agents), runs a single SPMD Bass program,
and gathers the full output.

v2 changes vs the fp32r baseline:
  - all matmul operands are bf16 (same 1 cycle/row PE throughput, half
    the DMA bytes / SBUF footprint / DVE eviction cost)
  - BatchNorm statistics are reduced per agent block as soon as that
    block's features are done, hiding the reductions under the next
    agent's conv; a warm-up AllReduce at t~0 absorbs collective setup
    and inter-core start skew off the critical path
  - activation tables (Sigmoid/Sqrt/Relu/Identity) are preloaded at
    kernel start while the first conv tiles stream in
  - the BN normalization is folded into the encoder weights
    (enc_w * rstd, bias - enc_w_s @ mu) so the post-collective serial
    chain is ~2us
  - the post-BN block is emitted interleaved across the 3 agents and
    the 2 Q-heads so the PE never waits on an eviction round-trip
  - outputs DMA out per agent block as soon as both heads finish
"""

import os
import numpy as np
import ml_dtypes

import concourse.bass as bass
import concourse.bacc as bacc
import concourse.tile as tile
from concourse import mybir
from concourse.bass_utils import run_bass_kernel_spmd

# ---- problem sizes (hardcoded per the task spec) ----
NAG, B, H, HEADS, AD = 3, 4096, 128, 8, 16
STATE, ACTD, SCAN, OUTF, HID = 48, 2, 256, 10, 256
EPS = 1e-5
NCORES = 8
BL = B // NCORES            # 512 rows per agent per core
R = NAG * BL                # 1536 rows per core
NB = BL                     # free-dim block = one agent block
P2 = 250                    # conv2 output positions
QT = 63                     # conv tiles of 4 positions (252 = 63*4)
NTOT = NAG * B

F32 = mybir.dt.float32
V2DT = os.environ.get("V2_DTYPE", "bf16")
BF16 = mybir.dt.bfloat16 if V2DT == "bf16" else mybir.dt.float32r
AX = mybir.AxisListType.X
AF = mybir.ActivationFunctionType
OP = mybir.AluOpType
BF16NP = ml_dtypes.bfloat16 if V2DT == "bf16" else np.float32


def _t1_parts(q):
    """conv1 tile q -> list of (t1 stack index, scan block index)."""
    if q <= 30:
        return [(q, 0)]
    if q == 31:
        return [(31, 0), (32, 1)]
    return [(q - 32, 1)]


def build_program():
    nc = bacc.Bacc(num_devices=NCORES)

    scan_t = nc.dram_tensor("scan_t", [SCAN, R], BF16, kind="ExternalInput")
    obs_t = nc.dram_tensor("obs_t", [STATE, R], BF16, kind="ExternalInput")
    acts_t = nc.dram_tensor("acts_t", [ACTD, R], BF16, kind="ExternalInput")
    t1_d = nc.dram_tensor("t1", [128, 33, 128], BF16, kind="ExternalInput")
    t2_d = nc.dram_tensor("t2", [128, 320], BF16, kind="ExternalInput")
    fc1w_d = nc.dram_tensor("fc1w", [128, QT, 256], BF16, kind="ExternalInput")
    fc2w_d = nc.dram_tensor("fc2w", [128, 2, 16], BF16, kind="ExternalInput")
    encw_d = nc.dram_tensor("encw", [128, 128], BF16, kind="ExternalInput")
    attw_d = nc.dram_tensor("attw", [128, 5, 128], BF16, kind="ExternalInput")
    hsum_d = nc.dram_tensor("hsum", [128, 8], BF16, kind="ExternalInput")
    hbc_d = nc.dram_tensor("hbc", [8, 128], BF16, kind="ExternalInput")
    mlpw_d = nc.dram_tensor("mlpw", [128, 2, 1152], BF16, kind="ExternalInput")
    bias_d = nc.dram_tensor("bias", [128, 20], F32, kind="ExternalInput")
    out_d = nc.dram_tensor("out", [2, R], F32, kind="ExternalOutput")

    with tile.TileContext(nc) as tc:
        with (
            tc.tile_pool(name="dram", bufs=1, space="DRAM") as dram,
            tc.tile_pool(name="cst", bufs=1) as cst,
            tc.tile_pool(name="ypool", bufs=4) as ypool,
            tc.tile_pool(name="opool", bufs=3) as opool,
            tc.tile_pool(name="xpool", bufs=4) as xpool,
            tc.tile_pool(name="wp2", bufs=3) as wp2,
            tc.tile_pool(name="qpool", bufs=3) as qpool,
            tc.tile_pool(name="othp", bufs=3) as othp,
            tc.tile_pool(name="attp", bufs=2 if V2DT != "bf16" else 4) as attp,
            tc.tile_pool(name="mlph", bufs=13 if V2DT != "bf16" else 26) as mlph,
        ):
            # ---- weight / input DMAs (program order ~ priority) ----
            biasb = cst.tile([128, 20], F32, tag="bias")
            nc.sync.dma_start(out=biasb, in_=bias_d[:])
            s0 = cst.tile([128, R], BF16, tag="s0")
            s1 = cst.tile([128, R], BF16, tag="s1")
            for p in range(0, 128, 32):
                nc.sync.dma_start(out=s0[p:p + 32, 0:NB],
                                  in_=scan_t[p:p + 32, 0:NB])
            t1c = []
            for k in range(4):
                n = 9 if k < 3 else 6
                t = cst.tile([128, n, 128], BF16, tag=f"t1c{k}")
                t1c.append(t)
            for j in range(3):
                nc.sync.dma_start(out=t1c[0][:, 3 * j:3 * j + 3, :],
                                  in_=t1_d[:, 3 * j:3 * j + 3, :])
            t2sb = cst.tile([128, 320], BF16, tag="t2")
            nc.sync.dma_start(out=t2sb, in_=t2_d[:])
            for k in range(1, 4):
                n = 9 if k < 3 else 6
                nc.sync.dma_start(out=t1c[k], in_=t1_d[:, 9 * k:9 * k + n, :])
            for p in range(0, 128, 64):
                nc.sync.dma_start(out=s1[p:p + 64, 0:NB],
                                  in_=scan_t[128 + p:128 + p + 64, 0:NB])
            fc1c = []
            for k in range(8):
                n = 8 if k < 7 else 7
                t = cst.tile([128, n, 256], BF16, tag=f"fc1c{k}")
                fc1c.append(t)
            nc.sync.dma_start(out=fc1c[0][:, 0:4, :], in_=fc1w_d[:, 0:4, :])
            nc.sync.dma_start(out=fc1c[0][:, 4:8, :], in_=fc1w_d[:, 4:8, :])
            nc.sync.dma_start(out=fc1c[1], in_=fc1w_d[:, 8:16, :])
            fc2w = cst.tile([128, 2, 16], BF16, tag="fc2w")
            nc.sync.dma_start(out=fc2w, in_=fc2w_d[:])
            nc.sync.dma_start(out=s0[:, NB:2 * NB], in_=scan_t[0:128, NB:2 * NB])
            nc.sync.dma_start(out=s1[:, NB:2 * NB],
                              in_=scan_t[128:256, NB:2 * NB])
            for k in range(2, 8):
                n = 8 if k < 7 else 7
                nc.sync.dma_start(out=fc1c[k], in_=fc1w_d[:, 8 * k:8 * k + n, :])
            nc.sync.dma_start(out=s0[:, 2 * NB:R], in_=scan_t[0:128, 2 * NB:R])
            nc.sync.dma_start(out=s1[:, 2 * NB:R],
                              in_=scan_t[128:256, 2 * NB:R])
            # BN feature rows in 32-aligned groups: obs 0:48, feats 64:74,
            # acts 96:98; everything else memset to zero.
            inps = cst.tile([128, R], BF16, tag="inps")
            nc.vector.memset(inps[:], 0.0)
            nc.sync.dma_start(out=inps[0:STATE, :], in_=obs_t[:])
            nc.sync.dma_start(out=inps[96:96 + ACTD, :], in_=acts_t[:])
            encw = cst.tile([128, 128], BF16, tag="encw")
            nc.sync.dma_start(out=encw, in_=encw_d[:])
            attw = cst.tile([128, 5, 128], BF16, tag="attw")
            nc.sync.dma_start(out=attw, in_=attw_d[:])
            hsum = cst.tile([128, 8], BF16, tag="hsum")
            nc.sync.dma_start(out=hsum, in_=hsum_d[:])
            hbc = cst.tile([8, 128], BF16, tag="hbc")
            nc.sync.dma_start(out=hbc, in_=hbc_d[:])
            mlpw = cst.tile([128, 2, 1152], BF16, tag="mlpw")
            nc.sync.dma_start(out=mlpw, in_=mlpw_d[:])

            # ---- warm-up: act tables + a dummy collective ----
            scr = cst.tile([128, 2], F32, tag="scr")
            nc.vector.memset(scr[:], 0.0)
            if os.environ.get("V2_PRELOAD", "1") == "1":
                scr2 = cst.tile([128, 1], F32, tag="scr2")
                nc.scalar.activation(scr2[:], scr[:, 0:1], AF.Sigmoid)
                nc.scalar.activation(scr2[:], scr[:, 0:1], AF.Sqrt)
                nc.scalar.activation(scr2[:], scr[:, 0:1], AF.Relu)
                nc.scalar.activation(scr2[:], scr[:, 0:1], AF.Identity)
            if os.environ.get("V2_DUMMY_CC", "1") == "1":
                ccd_in = dram.tile([1, 2], F32, tag="ccd_in")
                ccd_out = dram.tile([1, 2], F32, tag="ccd_out",
                                    addr_space="Shared")
                nc.gpsimd.dma_start(out=ccd_in[:], in_=scr[0:1, :])
                nc.gpsimd.collective_compute(
                    "AllReduce", OP.add,
                    replica_groups=[list(range(NCORES))],
                    ins=[ccd_in.opt()], outs=[ccd_out.opt()])

            saT = cst.tile([128, NAG, NB], BF16, tag="saT")
            keysT = cst.tile([128, NAG, NB], BF16, tag="keysT")
            valsT = cst.tile([128, NAG, NB], BF16, tag="valsT")
            outq1 = cst.tile([1, R], F32, tag="outq1")
            outq2 = cst.tile([1, R], F32, tag="outq2")
            statsP = cst.tile([128, 3], F32, tag="statsP")
            statsQ = cst.tile([128, 3], F32, tag="statsQ")
            sqscr = cst.tile([128, NB], F32, tag="sqscr")
            stats2 = cst.tile([128, 2], F32, tag="stats2")

            def evict_relu(dst, src_ps, bias_ap, use_act):
                if use_act:
                    nc.scalar.activation(dst, src_ps, AF.Relu, bias=bias_ap)
                else:
                    nc.vector.tensor_scalar(
                        out=dst, in0=src_ps, scalar1=bias_ap, scalar2=0.0,
                        op0=OP.add, op1=OP.max)

            # =========== conv stream (pre-BatchNorm), per agent block ========
            with (
                tc.tile_pool(name="ps_c1", bufs=2, space="PSUM") as ps_c1,
                tc.tile_pool(name="ps_c2", bufs=2, space="PSUM") as ps_c2,
                tc.tile_pool(name="ps_fc", bufs=2, space="PSUM") as ps_fc,
                tc.tile_pool(name="ps_f2", bufs=1, space="PSUM") as ps_f2,
            ):
                for g in range(NAG):
                    ctx_scope = nc.named_scope(f"conv{g}")
                    ctx_scope.__enter__()
                    col = bass.ts(g, NB)
                    fc_ps = [ps_fc.tile([128, NB], F32, tag="pfc",
                                        name=f"pfc{g}_{m}")
                             for m in range(2)]
                    y_tiles = {}

                    def conv2_and_fc1(t):
                        po = ps_c2.tile([128, NB], F32, tag="po")
                        if t < QT - 1:
                            rows = 128
                            nc.tensor.matmul(po, t2sb[:, 0:128], y_tiles[t][:],
                                             start=True, stop=False)
                            nc.tensor.matmul(po, t2sb[:, 128:256],
                                             y_tiles[t + 1][:],
                                             start=False, stop=True)
                            bcol = 1
                        else:
                            rows = 64
                            nc.tensor.matmul(po[0:64, :], t2sb[:, 256:320],
                                             y_tiles[t][:],
                                             start=True, stop=True)
                            bcol = 17
                        o2 = opool.tile([128, NB], BF16, tag="o2")
                        evict_relu(o2[0:rows, :], po[0:rows, :],
                                   biasb[0:rows, bcol:bcol + 1], t % 2 == 0)
                        for m in range(2):
                            lhs = fc1c[t // 8][0:rows, t % 8,
                                              128 * m:128 * m + 128]
                            nc.tensor.matmul(
                                fc_ps[m], lhs, o2[0:rows, :],
                                start=(t == 0), stop=(t == QT - 1),
                                skip_group_check=True)
                        del y_tiles[t]

                    for q in range(QT):
                        py = ps_c1.tile([128, NB], F32, tag="py")
                        parts = _t1_parts(q)
                        for i, (idx, sb_) in enumerate(parts):
                            src = (s0 if sb_ == 0 else s1)[:, col]
                            nc.tensor.matmul(py, t1c[idx // 9][:, idx % 9, :],
                                             src, start=(i == 0),
                                             stop=(i == len(parts) - 1))
                        yq = ypool.tile([128, NB], BF16, tag="y")
                        # first tiles of agent 0 evict on vector only, so the
                        # scalar engine can finish act-table preloads
                        use_act = (q % 2 == 1) and not (g == 0 and q < 8)
                        evict_relu(yq[:], py[:], biasb[:, 0:1], use_act)
                        y_tiles[q] = yq
                        if q >= 1:
                            conv2_and_fc1(q - 1)
                    conv2_and_fc1(QT - 1)

                    # fc1 eviction (+relu+fc_b1), fc2, feats -> inps[64:74]
                    x3 = []
                    for m in range(2):
                        xm = xpool.tile([128, NB], BF16, tag="x3")
                        nc.scalar.activation(xm[:], fc_ps[m][:], AF.Relu,
                                             bias=biasb[:, 2 + m:3 + m])
                        x3.append(xm)
                    pf = ps_f2.tile([OUTF, NB], F32, tag="pfeat")
                    for kb in range(2):
                        nc.tensor.matmul(pf, fc2w[:, kb, 0:OUTF], x3[kb][:],
                                         start=(kb == 0), stop=(kb == 1))
                    nc.scalar.activation(inps[64:64 + OUTF, col], pf[:],
                                         AF.Identity,
                                         bias=biasb[0:OUTF, 4:5])
                    # partial BN stats for this agent block (hidden under the
                    # next agent's conv)
                    nc.vector.reduce_sum(statsP[:, g:g + 1], inps[:, col],
                                         axis=AX)
                    if os.environ.get("V2_TTR", "0") == "1":
                        nc.vector.tensor_tensor_reduce(
                            out=sqscr[:], in0=inps[:, col], in1=inps[:, col],
                            scale=1.0, scalar=0.0, op0=OP.mult, op1=OP.add,
                            accum_out=statsQ[:, g:g + 1])
                    else:
                        nc.vector.tensor_mul(sqscr[:], inps[:, col],
                                             inps[:, col])
                        nc.vector.reduce_sum(statsQ[:, g:g + 1], sqscr[:],
                                             axis=AX)
                    ctx_scope.__exit__(None, None, None)

            # ================= BatchNorm statistics + AllReduce ==============
            bn_scope = nc.named_scope("bn")
            bn_scope.__enter__()
            if os.environ.get("V2_PRELOAD", "1") == "1":
                scr3 = cst.tile([1, 1], F32, tag="scr3")
                nc.scalar.activation(scr3[:], biasb[0:1, 18:19], AF.Sigmoid)
            nc.vector.reduce_sum(stats2[:, 0:1], statsP[:], axis=AX)
            nc.vector.reduce_sum(stats2[:, 1:2], statsQ[:], axis=AX)
            cc_in = dram.tile([128, 2], F32, tag="cc_in")
            cc_out = dram.tile([128, 2], F32, tag="cc_out",
                               addr_space="Shared")
            nc.gpsimd.dma_start(out=cc_in[:], in_=stats2[:])
            nc.gpsimd.collective_compute(
                "AllReduce", OP.add,
                replica_groups=[list(range(NCORES))],
                ins=[cc_in.opt()], outs=[cc_out.opt()])
            gst = cst.tile([128, 2], F32, tag="gst")
            nc.gpsimd.dma_start(out=gst[:], in_=cc_out[:])
            # var = gst1/N - (gst0/N)^2; sd = sqrt(var+eps)
            sqmu = cst.tile([128, 1], F32, tag="sqmu")
            nc.scalar.activation(sqmu[:], gst[:, 0:1], AF.Square,
                                 scale=1.0 / NTOT)
            var_ = cst.tile([128, 1], F32, tag="var_")
            nc.vector.scalar_tensor_tensor(
                out=var_[:], in0=gst[:, 1:2], scalar=1.0 / NTOT,
                in1=sqmu[:], op0=OP.mult, op1=OP.subtract)
            sd = cst.tile([128, 1], F32, tag="sd")
            nc.scalar.activation(sd[:], var_[:], AF.Sqrt,
                                 bias=biasb[:, 18:19])
            # fold BN into encoder: encw_s = encw / sd (per contraction
            # row); enc bias' = enc_b - encw_s.T @ mu
            rstd = cst.tile([128, 1], F32, tag="rstd")
            nc.vector.reciprocal(rstd[:], sd[:])
            encw_s = cst.tile([128, 128], BF16, tag="encw_s")
            nc.vector.tensor_scalar(out=encw_s[:], in0=encw[:],
                                    scalar1=rstd[:], scalar2=None,
                                    op0=OP.mult)
            mu_bf = cst.tile([128, 1], BF16, tag="mu_bf")
            nc.scalar.activation(mu_bf[:], gst[:, 0:1], AF.Identity,
                                 scale=1.0 / NTOT)
            encbias = cst.tile([128, 1], F32, tag="encbias")
            bn_scope.__exit__(None, None, None)

            # ======================= post-BN network =========================
            with (
                tc.tile_pool(name="ps_p", bufs=4, space="PSUM") as ps_p,
                tc.tile_pool(name="ps_l", bufs=1, space="PSUM") as ps_l,
                tc.tile_pool(name="ps_o", bufs=2, space="PSUM") as ps_o,
            ):
                post_scope = nc.named_scope("post")
                post_scope.__enter__()
                pmb = ps_l.tile([128, 1], F32, tag="pmb")
                nc.tensor.matmul(pmb, encw_s[:], mu_bf[:],
                                 start=True, stop=True)
                nc.vector.tensor_tensor(out=encbias[:], in0=biasb[:, 5:6],
                                        in1=pmb[:], op=OP.subtract)

                # --- sa = leaky(encw_s @ inps + encbias), all agents ---
                pe_ = []
                for g in range(NAG):
                    p = ps_p.tile([128, NB], F32, tag="pp", name=f"pe{g}")
                    nc.tensor.matmul(p, encw_s[:], inps[:, bass.ts(g, NB)],
                                     start=True, stop=True)
                    pe_.append(p)
                satmp = []
                for g in range(NAG):
                    tmp = wp2.tile([128, NB], F32, tag="tmp")
                    nc.scalar.activation(tmp[:], pe_[g][:], AF.Identity,
                                         bias=encbias[:])
                    nc.vector.scalar_tensor_tensor(
                        out=saT[:, g, :], in0=tmp[:], scalar=0.01, in1=tmp[:],
                        op0=OP.mult, op1=OP.max)
                    satmp.append(tmp)

                # --- keys first (they gate the attention entry) ---
                pv_, pq_, qT = [], [], []
                for g in range(NAG):
                    pk = ps_p.tile([128, NB], F32, tag="pp", name=f"pk{g}")
                    nc.tensor.matmul(pk, attw[:, 0, :], saT[:, g, :],
                                     start=True, stop=True)
                    nc.vector.tensor_scalar(out=keysT[:, g, :], in0=pk[:],
                                            scalar1=0.0, scalar2=None,
                                            op0=OP.add)
                for g in range(NAG):
                    pv = ps_p.tile([128, NB], F32, tag="pp", name=f"pv{g}")
                    nc.tensor.matmul(pv, attw[:, 1, :], saT[:, g, :],
                                     start=True, stop=True)
                    pq = ps_p.tile([128, NB], F32, tag="pp", name=f"pq{g}")
                    nc.tensor.matmul(pq, attw[:, 2 + g, :], saT[:, g, :],
                                     start=True, stop=True)
                    vtmp = wp2.tile([128, NB], F32, tag="tmp")
                    nc.scalar.activation(vtmp[:], pv[:], AF.Identity,
                                         bias=biasb[:, 6:7])
                    nc.vector.scalar_tensor_tensor(
                        out=valsT[:, g, :], in0=vtmp[:], scalar=0.01,
                        in1=vtmp[:], op0=OP.mult, op1=OP.max)
                    qg = qpool.tile([128, NB], BF16, tag="qt")
                    nc.scalar.activation(qg[:], pq[:], AF.Identity)
                    qT.append(qg)

                # --- attention per agent, MLP layer 1 interleaved ---
                h_cur = {}
                oth = []

                def mlp_l1(g, net):
                    h_prev = (saT[:, g, :], oth[g][:])
                    h_new = []
                    for mb in range(2):
                        pm = ps_p.tile([128, NB], F32, tag="pp",
                                       name=f"pm{g}_{net}_0_{mb}")
                        for kb in range(2):
                            c0 = kb * 256 + mb * 128
                            nc.tensor.matmul(
                                pm, mlpw[:, net, c0:c0 + 128],
                                h_prev[kb], start=(kb == 0), stop=(kb == 1))
                        hm = mlph.tile([128, NB], BF16, tag="h")
                        bcol = (7 if net == 0 else 12) + mb
                        evict_relu(hm[:], pm[:], biasb[:, bcol:bcol + 1],
                                   (2 * g + net + mb) % 2 == 0)
                        h_new.append(hm)
                    h_cur[(g, net)] = tuple(h_new)

                for g in range(NAG):
                    oa, ob = [o for o in range(NAG) if o != g]
                    dk = attp.tile([128, NB], BF16, tag="dk")
                    nc.vector.tensor_sub(dk[:], keysT[:, oa, :],
                                         keysT[:, ob, :])
                    dv = attp.tile([128, NB], BF16, tag="dv")
                    nc.vector.tensor_sub(dv[:], valsT[:, oa, :],
                                         valsT[:, ob, :])
                    prod = attp.tile([128, NB], BF16, tag="prod")
                    nc.vector.tensor_mul(prod[:], qT[g][:], dk[:])
                    pl = ps_l.tile([8, NB], F32, tag="pl")
                    nc.tensor.matmul(pl, hsum[:], prod[:],
                                     start=True, stop=True)
                    wa = attp.tile([8, NB], BF16, tag="wa")
                    nc.scalar.activation(wa[:], pl[:], AF.Sigmoid, scale=0.25)
                    pw = ps_p.tile([128, NB], F32, tag="pp", name=f"pw{g}")
                    nc.tensor.matmul(pw, hbc[:], wa[:], start=True, stop=True)
                    m1 = attp.tile([128, NB], F32, tag="m1")
                    nc.vector.tensor_mul(m1[:], pw[:], dv[:])
                    o = othp.tile([128, NB], BF16, tag="oth")
                    nc.vector.tensor_add(o[:], m1[:], valsT[:, ob, :])
                    oth.append(o)
                    mlp_l1(g, 0)
                    mlp_l1(g, 1)

                # --- MLP layer 2, then the output layer per agent ---
                for g in range(NAG):
                    for net in range(2):
                        h_prev = h_cur[(g, net)]
                        h_new = []
                        for mb in range(2):
                            pm = ps_p.tile([128, NB], F32, tag="pp",
                                           name=f"pm{g}_{net}_1_{mb}")
                            for kb in range(2):
                                c0 = 512 + kb * 256 + mb * 128
                                nc.tensor.matmul(
                                    pm, mlpw[:, net, c0:c0 + 128],
                                    h_prev[kb][:], start=(kb == 0),
                                    stop=(kb == 1))
                            hm = mlph.tile([128, NB], BF16, tag="h")
                            bcol = (9 if net == 0 else 14) + mb
                            evict_relu(hm[:], pm[:], biasb[:, bcol:bcol + 1],
                                       (2 * g + net + mb) % 2 == 0)
                            h_new.append(hm)
                        h_cur[(g, net)] = tuple(h_new)
                    col = bass.ts(g, NB)
                    for net in range(2):
                        po_ = ps_o.tile([1, NB], F32, tag="pout")
                        h_prev = h_cur[(g, net)]
                        for kb in range(2):
                            nc.tensor.matmul(
                                po_,
                                mlpw[:, net, 1024 + 64 * kb:1025 + 64 * kb],
                                h_prev[kb][:], start=(kb == 0), stop=(kb == 1))
                        bcol = 11 if net == 0 else 16
                        dst = (outq1 if net == 0 else outq2)[0:1, col]
                        nc.scalar.activation(dst, po_[:], AF.Identity,
                                             bias=biasb[0:1, bcol:bcol + 1])
                    nc.sync.dma_start(out=out_d[0:1, col],
                                      in_=outq1[0:1, col])
                    nc.sync.dma_start(out=out_d[1:2, col],
                                      in_=outq2[0:1, col])
                post_scope.__exit__(None, None, None)
    return nc


# ======================= host-side preparation ===========================

def _prep_shared(i):
    f32 = np.float32
    w1 = np.asarray(i["conv_w1"], f32)[:, 0, :]           # [32, 5]
    w2 = np.asarray(i["conv_w2"], f32)                    # [32, 32, 3]
    fw1 = np.asarray(i["fc_w1"], f32)                     # [256, 8000]
    fw2 = np.asarray(i["fc_w2"], f32)                     # [10, 256]
    encw_ = np.asarray(i["enc_w"], f32)                   # [128, 60]
    Wk = np.asarray(i["Wk"], f32)
    Wv = np.asarray(i["Wv"], f32)
    Wq = np.asarray(i["Wq"], f32)

    t1 = np.zeros((128, 33, 128), f32)
    for idx in range(32):
        r0 = 4 * idx if idx < 31 else 124
        for dp in range(4):
            for j in range(5):
                r = r0 + dp + j
                if r < 128:
                    t1[r, idx, dp::4] = w1[:, j]
    for dp in range(4):
        for r in range(4):
            j = r + 4 - dp
            if 0 <= j < 5:
                t1[r, 32, dp::4] = w1[:, j]

    t2 = np.zeros((128, 320), f32)
    for dp in range(4):
        for j in range(3):
            e = dp + j
            for cp in range(32):
                if e < 4:
                    t2[4 * cp + e, dp:128:4] = w2[:, cp, j]
                else:
                    t2[4 * cp + (e - 4), 128 + dp:256:4] = w2[:, cp, j]
    for dp in range(2):
        for j in range(3):
            e = dp + j
            for cp in range(32):
                t2[4 * cp + e, 256 + dp:320:2] = w2[:, cp, j]

    fc1w = np.zeros((128, QT, 256), f32)
    for q in range(QT - 1):
        for dp in range(4):
            p = 4 * q + dp
            fc1w[dp::4, q, :] = fw1[:, p::P2].T
    for dp in range(2):
        fc1w[dp:64:2, QT - 1, :] = fw1[:, (248 + dp)::P2].T

    fc2w = np.zeros((128, 2, 16), f32)
    for kb in range(2):
        fc2w[:, kb, 0:OUTF] = fw2[:, 128 * kb:128 * kb + 128].T

    encw_full = np.zeros((128, 128), f32)
    encw_full[0:STATE, :] = encw_.T[0:STATE, :]            # obs rows
    encw_full[64:64 + OUTF, :] = encw_.T[50:60, :]         # feats rows
    encw_full[96:96 + ACTD, :] = encw_.T[48:50, :]         # acts rows

    attw = np.zeros((128, 5, 128), f32)
    attw[:, 0, :] = Wk.reshape(128, H).T
    attw[:, 1, :] = Wv.reshape(128, H).T
    for g in range(NAG):
        attw[:, 2 + g, :] = Wq[g].reshape(128, H).T

    hsum = np.kron(np.eye(8, dtype=f32), np.ones((16, 1), f32))  # [128, 8]
    hbc = np.ascontiguousarray(hsum.T)                           # [8, 128]

    mlpw = np.zeros((128, 2, 1152), f32)
    for net, pre in enumerate(("q1", "q2")):
        mw1 = np.asarray(i[pre + "_w1"], f32)
        mw2 = np.asarray(i[pre + "_w2"], f32)
        mw3 = np.asarray(i[pre + "_w3"], f32)
        for kb in range(2):
            mlpw[:, net, kb * 256:(kb + 1) * 256] = \
                mw1[:, 128 * kb:128 * kb + 128].T
            mlpw[:, net, 512 + kb * 256:512 + (kb + 1) * 256] = \
                mw2[:, 128 * kb:128 * kb + 128].T
            mlpw[:, net, 1024 + 64 * kb] = mw3[0, 128 * kb:128 * kb + 128]

    bias = np.zeros((128, 20), f32)
    bias[:, 0] = np.repeat(np.asarray(i["conv_b1"], f32), 4)
    bias[:, 1] = np.repeat(np.asarray(i["conv_b2"], f32), 4)
    bias[:, 2] = np.asarray(i["fc_b1"], f32)[0:128]
    bias[:, 3] = np.asarray(i["fc_b1"], f32)[128:256]
    bias[0:OUTF, 4] = np.asarray(i["fc_b2"], f32)
    bias[:, 5] = np.asarray(i["enc_b"], f32)
    bias[:, 6] = np.asarray(i["bv"], f32).reshape(128)
    for net, pre in enumerate(("q1", "q2")):
        b1 = np.asarray(i[pre + "_b1"], f32)
        b2 = np.asarray(i[pre + "_b2"], f32)
        b3 = np.asarray(i[pre + "_b3"], f32)
        c0 = 7 if net == 0 else 12
        bias[:, c0] = b1[0:128]
        bias[:, c0 + 1] = b1[128:256]
        bias[:, c0 + 2] = b2[0:128]
        bias[:, c0 + 3] = b2[128:256]
        bias[0, 11 if net == 0 else 16] = b3[0]
    bias[0:64, 17] = np.repeat(np.asarray(i["conv_b2"], f32), 2)
    bias[:, 18] = EPS

    bf = BF16NP
    return {
        "t1": t1.astype(bf), "t2": t2.astype(bf),
        "fc1w": fc1w.astype(bf), "fc2w": fc2w.astype(bf),
        "encw": encw_full.astype(bf), "attw": attw.astype(bf),
        "hsum": hsum.astype(bf), "hbc": hbc.astype(bf),
        "mlpw": mlpw.astype(bf), "bias": bias,
    }


def _shard(arr, c):
    out = np.empty((R, arr.shape[1]), np.float32)
    for g in range(NAG):
        out[g * BL:(g + 1) * BL] = arr[g * B + c * BL: g * B + (c + 1) * BL]
    return np.ascontiguousarray(out.T).astype(BF16NP)


_CACHE = {}


def _get_prog():
    if "nc" not in _CACHE:
        nc = build_program()
        nc.finalize()
        _CACHE["nc"] = nc
    return _CACHE["nc"]


def _make_in_maps(inputs):
    shared = _prep_shared(inputs)
    obs = np.asarray(inputs["obs"], np.float32)
    acts = np.asarray(inputs["acts"], np.float32)
    scan = np.asarray(inputs["scan"], np.float32)
    in_maps = []
    for c in range(NCORES):
        m = dict(shared)
        m["scan_t"] = _shard(scan, c)
        m["obs_t"] = _shard(obs, c)
        m["acts_t"] = _shard(acts, c)
        in_maps.append(m)
    return in_maps


def _gather(results):
    q = np.empty((2, NAG, B), np.float32)
    for c, r in enumerate(results):
        o = np.asarray(r["out"]).reshape(2, NAG, BL)
        q[:, :, c * BL:(c + 1) * BL] = o
    q1 = np.ascontiguousarray(q[0].reshape(NTOT, 1))
    q2 = np.ascontiguousarray(q[1].reshape(NTOT, 1))
    return q1, q2


def kernel(**inputs):
    nc = _get_prog()
    in_maps = _make_in_maps(inputs)
    if os.environ.get("KERNEL_BACKEND") == "sim":
        from concourse import bass_interp
        sim = bass_interp.MultiCoreSim(nc, NCORES)
        for c in range(NCORES):
            for k, v in in_maps[c].items():
                sim.cores[c].tensor(k)[:] = v
        sim.simulate()
        results = [{"out": np.array(sim.cores[c].tensor("out"))}
                   for c in range(NCORES)]
        return _gather(results)
    res = run_bass_kernel_spmd(nc, in_maps, list(range(NCORES)))
    return _gather(res.results)


def kernel_profiled(**inputs):
    """Like kernel() but NTFF-traced; returns ((q1, q2), exec_time_ns)."""
    _install_ntff_hook()
    nc = _get_prog()
    in_maps = _make_in_maps(inputs)
    res = run_bass_kernel_spmd(nc, in_maps, list(range(NCORES)), trace=True)
    return _gather(res.results), res.exec_time_ns


def _install_ntff_hook():
    """Provide antenv.axon_hooks (absent in this image) and register the
    ctypes NTFF profile hook against libaxon_pjrt.so."""
    import sys
    import types
    import ctypes
    import contextlib

    if "antenv.axon_hooks" not in sys.modules:
        mod = types.ModuleType("antenv.axon_hooks")
        mod._hook = None
        mod.set_axon_ntff_profile_hook = lambda h: setattr(mod, "_hook", h)
        mod.get_axon_ntff_profile_hook = lambda: mod._hook
        sys.modules["antenv.axon_hooks"] = mod
        import antenv
        antenv.axon_hooks = mod
    mod = sys.modules["antenv.axon_hooks"]
    if mod.get_axon_ntff_profile_hook() is not None:
        return
    so_path = "/opt/axon/libaxon_pjrt.so"
    lib = ctypes.CDLL(so_path)
    if not hasattr(lib, "axon_start_nrt_profile"):
        return
    lib.axon_start_nrt_profile.argtypes = [
        ctypes.POINTER(ctypes.c_int64), ctypes.c_size_t]
    lib.axon_start_nrt_profile.restype = ctypes.c_int64
    lib.axon_stop_nrt_profile.argtypes = [ctypes.c_char_p]
    lib.axon_stop_nrt_profile.restype = ctypes.c_int64

    @contextlib.contextmanager
    def _hook(output_dir, device_ids):
        import jax
        jax.devices()
        if device_ids:
            ids = (ctypes.c_int64 * len(device_ids))(*device_ids)
            rc = lib.axon_start_nrt_profile(ids, len(device_ids))
        else:
            rc = lib.axon_start_nrt_profile(None, 0)
        if rc != 0:
            raise RuntimeError(f"axon_start_nrt_profile rc={rc}")
        try:
            yield
        finally:
            n = lib.axon_stop_nrt_profile(str(output_dir).encode())
            if n < 0:
                raise RuntimeError(f"axon_stop_nrt_profile rc={n}")

    mod.set_axon_ntff_profile_hook(_hook)
